# revision 16
# baseline (speedup 1.0000x reference)
"""ABCNN1 Trainium2 kernel (8 NeuronCores, data-parallel over batch).

Computes, for xa/xb [B,S,D]:
  d2   = |xa_s|^2 + |xb_t|^2 - 2 xa.xb^T          [B,S,S]
  attn = 1/(sqrt(d2)+1)
  xa_attn = attn   @ weight ; xb_attn = attn^T @ weight
  img_a = [xa^T ; xa_attn^T]  (2*D x S), img_b likewise
  out_a = relu(conv1d_{w=3,same}(img_a, conv_w) + conv_b)   [B,O,S]

Sharding: batch 32 -> 4 per core (data parallel, params replicated).

Key restructurings vs the straightforward mapping (HW time is all PE):
  - the attention GEMMs and the attn conv channels fuse into the conv:
      conv_ch1_a[o,s] = sum_w sum_t Mw[w,o,t] attnT[t, s+w-1]
      conv_ch1_b[o,t] = sum_w sum_s Mw[w,o,s] attn [s, t+w-1]
    with Mw[w,o,t] = sum_d conv_w[o,1,d,w] weight[t,d] precomputed on
    host, run as fp8 DoubleRow matmuls straight into the conv PSUM
    (ch1 carries ~1.2%% of output amplitude -> fp8 invisible).
  - the x-channel conv (the dominant PE cost) is Winograd F(2,3):
      m_i[o,u] = sum_d wtil_i[o,d] dtil_i[d,u]    (4 GEMMs, N=256)
      y[2u] = m0+m1+m2 ; y[2u+1] = m1-m2-m3
    dtil ships from host in bf16 (comp-major so conv starts as chains
    land; the 1/2 factors of B^T are folded into dtil_1/2); wtil derives
    on-chip from the direct-conv weights (saves startup HBM).  This cuts
    conv x-channel matmul cycles 1.5x (18x512-col -> 24x256-col passes).
  - the attn channel folds into Winograd regions 0/3 by splitting its
    direct conv into even/odd output columns (stride-2 fp8 streams, the
    odd half with negated weights), so y_even = r0+r1+r2 and
    y_odd = r1-r2-r3 pick it up with no extra PSUM bank or DVE work.
  - inverse transform = 4 DVE scalar_tensor_tensor per (image, o-tile),
    then ACT relu+bias with stride-2 dst columns.
  - distance GEMM bf16->fp8 DoubleRow (x16 both sides); nb folds in via
    DVE stt, na via the sqrt-pass ACT bias; attn = 1/(1+sqrt(d2)) via
    ACT Sqrt + DVE reciprocal_approx_fast; chain emitted stage-major so
    2 PSUM banks suffice without stalling the PE queue.
  - attn^T (needed for image a's fused channel) via PE fp8 transpose.
  - startup is DMA-bound: per-queue FIFO encodes priority (xt8(0) ->
    params -> dtil_b(0) -> xt8(1) -> ...), descriptors <=2KB so in-
    flight chains don't slow concurrent matmuls.
"""

import numpy as np
import ml_dtypes

import concourse.bass as bass
from concourse import bacc
import concourse.mybir as mybir
import concourse.tile as tile
from concourse.bass_utils import run_bass_kernel_spmd
from concourse.masks import make_identity

AF = mybir.ActivationFunctionType
ALU = mybir.AluOpType
BF = mybir.dt.bfloat16
F32 = mybir.dt.float32
F8 = mybir.dt.float8e4
PM = mybir.MatmulPerfMode

B, S, D, O, W = 32, 512, 768, 256, 3
NCORES = 8
BPC = B // NCORES          # batches per core
P = 128
KD = D // P                # 6   d-tiles
KS = S // P                # 4   s-tiles
MO = O // P                # 2   o-tiles
U = S // 2                 # 256 winograd tiles (2 outputs each)
COL0 = 1                   # first data column (col 0 and col 513 are zero)
AIMG_W = 528               # fp8 attn image width (16B-aligned row stride)
DTW = 4 * KD * U           # dtil flat width per partition (bf16)


def _build_nc() -> bass.Bass:
    nc = bacc.Bacc()
    xt8a_d = nc.declare_dram_parameter("xt8a", [BPC, P, KD * S], F8, isOutput=False)
    xt8b_d = nc.declare_dram_parameter("xt8b", [BPC, P, KD * S], F8, isOutput=False)
    dta_d = nc.declare_dram_parameter("dta", [BPC, P, DTW], BF, isOutput=False)
    dtb_d = nc.declare_dram_parameter("dtb", [BPC, P, DTW], BF, isOutput=False)
    # nb-row broadcast and the na bias packed in one tensor (bf16 is plenty:
    # attn carries ~1.2% of the output amplitude)
    nab_d = nc.declare_dram_parameter("nab", [BPC, P, S + KS], BF, isOutput=False)
    cwtw_d = nc.declare_dram_parameter("cwtw", [P, 4 * KD * O], BF, isOutput=False)
    mw8_d = nc.declare_dram_parameter("mw8", [P, 2 * KS * W * O], F8, isOutput=False)
    cb_d = nc.declare_dram_parameter("cb", [P, MO], F32, isOutput=False)
    out_d = nc.declare_dram_parameter("out", [2, BPC, O, S], BF, isOutput=True)

    with tile.TileContext(nc) as tc:
        with (
            tc.tile_pool(name="const", bufs=1) as constp,
            tc.tile_pool(name="img", bufs=3) as imgp,
            tc.tile_pool(name="attn", bufs=3) as attnp,
            tc.tile_pool(name="scr", bufs=3) as scrp,
            tc.tile_pool(name="chain", bufs=8) as chainp,
            tc.tile_pool(name="inv", bufs=3) as invp,
            tc.tile_pool(name="outp", bufs=3) as outp,
            tc.tile_pool(name="psumd", bufs=3, space="PSUM") as psumdp,
            tc.tile_pool(name="psum", bufs=2, space="PSUM") as psump,
            tc.tile_pool(name="psumt", bufs=1, space="PSUM") as psumtp,
        ):
            # ---- persistent (replicated) operands ----
            cwtw_sb = constp.tile([P, 4, KD, O], BF)     # winograd wtil comps
            mw8pm_sb = constp.tile([P, 2, KS, W, O], F8)  # [+mw8, -mw8]
            cb_sb = constp.tile([P, MO], F32)
            ident8 = constp.tile([P, P], F8)
            make_identity(nc, ident8[:])
            # warm the ACT function tables (Sqrt/Relu) at t=0: the lazy
            # ACT_TABLE_LOAD (1.3us) otherwise lands on batch-0's attn chain
            warm = constp.tile([1, 2], F32)
            nc.gpsimd.memset(warm[:], 1.0)
            nc.scalar.activation(warm[:, 0:1], warm[:, 1:2], AF.Sqrt)
            nc.scalar.activation(warm[:, 1:2], warm[:, 0:1], AF.Relu)

            queues = [nc.sync, nc.gpsimd]

            def spread_dma(dst_f, src_f, nch, q0=0):
                w_ = dst_f.shape[-1] // nch
                for c in range(nch):
                    queues[(q0 + c) % len(queues)].dma_start(
                        dst_f[:, c * w_ : (c + 1) * w_],
                        src_f[:, c * w_ : (c + 1) * w_],
                    )

            def alloc_state():
                st = {}
                st["xt8_a"] = attnp.tile([P, KD, S], F8, tag="xt8_a",
                                         name="xt8_a")
                st["xt8_b"] = attnp.tile([P, KD, S], F8, tag="xt8_b",
                                         name="xt8_b")
                st["nab"] = scrp.tile([P, S + KS], BF, tag="nab", name="nab")
                st["dt_a"] = imgp.tile([P, DTW], BF, tag="dt_a", name="dt_a")
                st["dt_b"] = imgp.tile([P, DTW], BF, tag="dt_b", name="dt_b")
                st["attn_img"] = attnp.tile([P, KS, AIMG_W], F8,
                                            tag="attn_img", name="attn_img")
                st["attnT_img"] = attnp.tile([P, KS, AIMG_W], F8,
                                             tag="attnT_img",
                                             name="attnT_img")
                return st

            def pad_memsets(st):
                # fp8 attn images written later by the ACT chain / PE
                # transposes; zero the pad columns now.
                for aimg in (st["attn_img"], st["attnT_img"]):
                    nc.gpsimd.memset(aimg[:, :, 0:1], 0.0)
                    nc.gpsimd.memset(aimg[:, :, COL0 + S : COL0 + S + 1], 0.0)

            def load_dt(dt_sb, dt_src, qs):
                # 6 chains of 2KB in comp-consumption order 1,2,0,3 over
                # the given queues
                hw = DTW // 8
                for i, (c0, c1) in enumerate(
                        ((2, 3), (4, 5), (0, 1), (6, 7))):
                    q = qs[i % len(qs)]
                    q.dma_start(dt_sb[:, c0 * hw : (c1 + 1) * hw],
                                dt_src[:, c0 * hw : (c1 + 1) * hw])

            def startup_loads():
                """Batches 0/1 + params, priority-ordered via per-queue
                FIFO; wtil/mw8n derive on gpsimd at startup."""
                sts = [alloc_state(), alloc_state()]
                # pad memsets first: gpsimd runs them before its DMA-issue
                # backlog can block the sequencer
                pad_memsets(sts[0])
                pad_memsets(sts[1])
                # wave 0: nabs (tiny, needed by the first chain stt at
                # ~15us) then batch-0 distance operands
                nc.sync.dma_start(sts[0]["nab"][:], nab_d[0])
                nc.gpsimd.dma_start(sts[1]["nab"][:], nab_d[1])
                spread_dma(sts[0]["xt8_a"].rearrange("p kd s -> p (kd s)"),
                           xt8a_d[0], 3, q0=0)
                spread_dma(sts[0]["xt8_b"].rearrange("p kd s -> p (kd s)"),
                           xt8b_d[0], 3, q0=1)
                # params + batch-0 dtil_b interleaved per conv-comp
                # consumption order (1,2,0,3), on sync+gpsimd only: scalar
                # must carry NO DMA -- issues wait for ring slots and the
                # ACT chains queue behind them
                cwtw_f = cwtw_sb.rearrange("p c kd o -> p (c kd o)")
                w4 = 4 * KD * O // 4
                hw = DTW // 8
                for i, c in enumerate((1, 2, 0, 3)):
                    queues[i % 2].dma_start(
                        cwtw_f[:, c * w4 : (c + 1) * w4],
                        cwtw_d[:, c * w4 : (c + 1) * w4])
                    q = queues[(i + 1) % 2]
                    q.dma_start(sts[0]["dt_b"][:, 2 * c * hw : (2 * c + 2) * hw],
                                dtb_d[0][:, 2 * c * hw : (2 * c + 2) * hw])
                mw8_f = mw8pm_sb.rearrange("p pm tt w o -> p (pm tt w o)")
                hm = KS * W * O
                nc.sync.dma_start(mw8_f[:, 0:hm], mw8_d[:, 0:hm])
                nc.gpsimd.dma_start(mw8_f[:, hm:], mw8_d[:, hm:])
                nc.gpsimd.dma_start(cb_sb[:], cb_d[:])
                # wave 1: batch-1 xt8
                spread_dma(sts[1]["xt8_a"].rearrange("p kd s -> p (kd s)"),
                           xt8a_d[1], 3, q0=1)
                spread_dma(sts[1]["xt8_b"].rearrange("p kd s -> p (kd s)"),
                           xt8b_d[1], 3, q0=0)
                # wave 2: batch-0 dtil_a
                load_dt(sts[0]["dt_a"], dta_d[0], [nc.sync, nc.gpsimd])
                return sts

            def startup_loads_late(sts):
                # batch-1 dtils: emitted after batch-0 conv so the load
                # queues' sequencers aren't saturated during startup
                load_dt(sts[1]["dt_b"], dtb_d[1], [nc.sync, nc.gpsimd])
                load_dt(sts[1]["dt_a"], dta_d[1], [nc.gpsimd, nc.sync])

            def stage_load(b):
                """Steady-state loads for batch b>=2."""
                st = alloc_state()
                nc.sync.dma_start(st["nab"][:], nab_d[b])
                spread_dma(st["xt8_a"].rearrange("p kd s -> p (kd s)"),
                           xt8a_d[b], 3, q0=0)
                spread_dma(st["xt8_b"].rearrange("p kd s -> p (kd s)"),
                           xt8b_d[b], 3, q0=1)
                load_dt(st["dt_b"], dtb_d[b], [nc.sync, nc.gpsimd])
                load_dt(st["dt_a"], dta_d[b], [nc.gpsimd, nc.sync])
                pad_memsets(st)
                return st

            def stage_dist_mms(b, st):
                """Distance GEMM mms + the stts that free their PSUM."""
                xt8_a, xt8_b = st["xt8_a"], st["xt8_b"]
                nab = st["nab"]
                wkms = []
                for ms in range(KS):
                    ps = psumdp.tile([P, S], F32, tag="ps", name="ps")
                    for k2 in range(KD // 2):
                        nc.tensor.matmul(
                            ps[:],
                            xt8_a[:, 2 * k2 : 2 * k2 + 2, ms * P : (ms + 1) * P],
                            xt8_b[:, 2 * k2 : 2 * k2 + 2, :],
                            start=(k2 == 0),
                            stop=(k2 == KD // 2 - 1),
                            perf_mode=PM.DoubleRow,
                        )
                    # wkm = -2/256*ps + (nb-768); sqrt adds na+768 as bias:
                    # d2 = na + nb - 2*g  (d2 >= ~900 for gaussian data; the
                    # reference's 1e-12 clamp can never bind -> no relu)
                    wkm = chainp.tile([P, S], F32, tag="wkm", name="wkm")
                    nc.vector.scalar_tensor_tensor(
                        wkm[:], ps[:], -2.0 / 256.0, nab[:, 0:S],
                        ALU.mult, ALU.add,
                    )
                    wkms.append(wkm)
                return wkms

            def stage_dist_chain(b, st, wkms):
                """sqrt/add/recip/copy sweeps -> attn_img fp8."""
                nab = st["nab"]
                attn_img = st["attn_img"]
                sms = []
                for ms in range(KS):
                    sm = chainp.tile([P, S], F32, tag="sm", name="sm")
                    nc.scalar.activation(
                        sm[:], wkms[ms][:], AF.Sqrt,
                        bias=nab[:, S + ms : S + ms + 1], scale=1.0,
                    )
                    sms.append(sm)
                for ms in range(KS):
                    nc.vector.tensor_scalar_add(wkms[ms][:], sms[ms][:], 1.0)
                for ms in range(KS):
                    nc.vector.reciprocal_approx_fast(sms[ms][:], wkms[ms][:])
                for ms in range(KS):
                    nc.scalar.activation(
                        attn_img[:, ms, COL0 : COL0 + S], sms[ms][:],
                        AF.Copy, scale=128.0,
                    )

            def stage_dist(b, st):
                stage_dist_chain(b, st, stage_dist_mms(b, st))

            # attn-channel tap -> (w, col0, parity) of the stride-2 view;
            # even outputs accumulate into region 0, odd (negated weights)
            # into region 3, so the inverse transform picks them up free.
            ATAPS = {0: [(0, 0, 0), (1, 0, 1), (2, 1, 0)],
                     3: [(0, 0, 1), (1, 1, 0), (2, 1, 1)]}
            REG_ORDER = (0, 3, 1, 2)

            def conv_image(b, ii, dt, rimg, mos=None, tail=False):
                """Winograd conv for one image: per o-tile 24 bf16 256-col
                passes (x channel comps) + 12 fp8 DR 256-col passes (fused
                attn channel, even/odd split) into one [P,4,256] PSUM
                tile; inverse transform on DVE, relu+bias on ACT."""
                osb = outp.tile([P, MO, S], BF, tag="osb", name="osb")
                osb_v = osb.rearrange("p m (c two) -> p m c two", two=2)
                rv = rimg.rearrange("p k (c two) -> p k c two", two=2)
                for mo in (range(MO) if mos is None else mos):
                    ps = psump.tile([P, 4, U], F32, tag="ps", name="ps")
                    for comp in (1, 2, 0, 3):
                        for kd in range(KD):
                            nc.tensor.matmul(
                                ps[:, comp, :],
                                cwtw_sb[:, comp, kd, mo * P : (mo + 1) * P],
                                dt[:, (comp * KD + kd) * U :
                                      (comp * KD + kd + 1) * U],
                                start=(kd == 0),
                                stop=(comp not in ATAPS and kd == KD - 1),
                            )
                        if comp in ATAPS:
                            msb = mw8pm_sb[:, 0 if comp == 0 else 1]
                            idx = 0
                            for w, c0, par in ATAPS[comp]:
                                for k2 in range(KS // 2):
                                    idx += 1
                                    nc.tensor.matmul(
                                        ps[:, comp, :],
                                        msb[:, 2 * k2 : 2 * k2 + 2, w,
                                            mo * P : (mo + 1) * P],
                                        rv[:, 2 * k2 : 2 * k2 + 2,
                                           c0 : c0 + U, par],
                                        start=False,
                                        stop=(idx == KS // 2 * W),
                                        perf_mode=PM.DoubleRow,
                                    )
                    # inverse transform: y_even = r0+r1+r2, y_odd =
                    # r1-r2-r3.  PSUM has one DVE read port, so each op
                    # reads at most one PSUM operand (r1 staged to SBUF).
                    t1 = invp.tile([P, U], F32, tag="t1", name="t1")
                    te = invp.tile([P, U], F32, tag="te", name="te")
                    to = invp.tile([P, U], F32, tag="to", name="to")
                    ye = invp.tile([P, U], BF, tag="ye", name="ye")
                    yo = invp.tile([P, U], BF, tag="yo", name="yo")
                    nc.vector.tensor_scalar_mul(t1[:], ps[:, 1, :], 1.0)
                    nc.vector.scalar_tensor_tensor(
                        te[:], ps[:, 0, :], 1.0, t1[:], ALU.mult, ALU.add)
                    nc.vector.scalar_tensor_tensor(
                        ye[:], ps[:, 2, :], 1.0, te[:], ALU.mult, ALU.add)
                    nc.vector.scalar_tensor_tensor(
                        to[:], ps[:, 2, :], -1.0, t1[:], ALU.mult, ALU.add)
                    nc.vector.scalar_tensor_tensor(
                        yo[:], ps[:, 3, :], -1.0, to[:], ALU.mult, ALU.add)
                    nc.scalar.activation(
                        osb_v[:, mo, :, 0], ye[:], AF.Relu,
                        bias=cb_sb[:, mo : mo + 1], scale=1.0 / 4096.0,
                    )
                    nc.scalar.activation(
                        osb_v[:, mo, :, 1], yo[:], AF.Relu,
                        bias=cb_sb[:, mo : mo + 1], scale=1.0 / 4096.0,
                    )
                    # out stores on sync+gpsimd (scalar stays the ACT
                    # engine); 4 chains for the final image so the kernel
                    # tail drains fast
                    oq = ([nc.scalar, nc.sync, nc.gpsimd, nc.scalar]
                          if tail else [nc.sync, nc.gpsimd])
                    hq = S // len(oq)
                    for c, q in enumerate(oq):
                        q.dma_start(
                            out_d[ii, b, mo * P : (mo + 1) * P,
                                  c * hq : (c + 1) * hq],
                            osb[:, mo, c * hq : (c + 1) * hq],
                        )

            def stage_transp(b, st):
                # ---- attn^T via PE fp8 transpose ----
                attn_img, attnT_img = st["attn_img"], st["attnT_img"]
                for tt in range(KS):
                    # fp8 transpose mode writes with element step 2
                    pst = psumtp.tile([P, 2 * S], F8, tag="ps_t", name="pst")
                    pstv = pst.rearrange("p (j two) -> p j two", two=2)
                    for ss in range(KS):
                        nc.tensor.transpose(
                            pstv[:, ss * P : (ss + 1) * P, 0],
                            attn_img[:, ss, COL0 + tt * P : COL0 + (tt + 1) * P],
                            ident8[:],
                        )
                    nc.scalar.copy(
                        attnT_img[:, tt, COL0 : COL0 + S], pstv[:, :, 0]
                    )

            def stage_rest(b, st, tail=False):
                # image b first: its fused channel reads attn directly (no
                # dependency on the transposes below)
                conv_image(b, 1, st["dt_b"], st["attn_img"])
                stage_transp(b, st)
                conv_image(b, 0, st["dt_a"], st["attnT_img"], tail=tail)

            # software-pipelined emission: batch b's dist matmuls sit
            # between batch b-1's dist and rest stages, so the PE always
            # has conv work while b's ACT/DVE attn chain runs.
            states = [None] * BPC
            sts01 = startup_loads()
            states[0], states[1] = sts01[0], sts01[1]
            wk0 = stage_dist_mms(0, states[0])
            wk1 = stage_dist_mms(1, states[1])
            stage_dist_chain(0, states[0], wk0)
            stage_dist_chain(1, states[1], wk1)
            conv_image(0, 1, states[0]["dt_b"], states[0]["attn_img"])
            startup_loads_late(sts01)
            stage_transp(0, states[0])
            states[2] = stage_load(2)
            conv_image(0, 0, states[0]["dt_a"], states[0]["attnT_img"])
            stage_dist(2, states[2])
            states[3] = stage_load(3)
            stage_rest(1, states[1])
            stage_dist(3, states[3])
            stage_rest(2, states[2])
            stage_rest(BPC - 1, states[BPC - 1], tail=True)
    return nc


def _in_maps(xa, xb, weight, conv_w, conv_b):
    bf16 = ml_dtypes.bfloat16
    f8 = ml_dtypes.float8_e4m3
    xa32 = np.asarray(xa, np.float32)
    xb32 = np.asarray(xb, np.float32)
    w32 = np.asarray(weight, np.float32)
    cw32 = np.asarray(conv_w, np.float32)

    # x^T layouts, partition-major: [B, P, KD, S] with d = kd*128 + p
    xaT = np.ascontiguousarray(
        xa32.transpose(0, 2, 1).reshape(B, KD, P, S).transpose(0, 2, 1, 3)
    )
    xbT = np.ascontiguousarray(
        xb32.transpose(0, 2, 1).reshape(B, KD, P, S).transpose(0, 2, 1, 3)
    )
    xt8a = (xaT * 16.0).astype(f8).reshape(B, P, KD * S)
    xt8b = (xbT * 16.0).astype(f8).reshape(B, P, KD * S)

    # Winograd F(2,3) input-transform comps from the zero-padded image
    # (col c = x[c-1]); the G-matrix 1/2 is folded into comps 1/2.
    # dt layout: [B, P, 4, KD, U] flat -> [B, P, DTW]
    def make_dt(xT):  # xT: [B, D, S] f32
        img = np.zeros((B, D, 516), np.float32)
        img[:, :, 1:513] = xT
        e0 = img[:, :, 0:512:2]
        o0 = img[:, :, 1:513:2]
        e1 = img[:, :, 2:514:2]
        o1 = img[:, :, 3:515:2]
        dt = np.stack(
            [e0 - e1, (o0 + e1) * 0.5, (e1 - o0) * 0.5, o0 - o1], axis=1
        )  # [B, 4, D, U]
        return np.ascontiguousarray(
            dt.reshape(B, 4, KD, P, U).transpose(0, 3, 1, 2, 4)
        ).astype(bf16).reshape(B, P, DTW)

    dta = make_dt(xa32.transpose(0, 2, 1))
    dtb = make_dt(xb32.transpose(0, 2, 1))

    # norms (f32): na bias = na + 768 as [B, P, KS]; nb row = nb - 768
    na = np.einsum("bsd,bsd->bs", xa32, xa32)
    nb = np.einsum("bsd,bsd->bs", xb32, xb32)
    nab_h = np.empty((B, P, S + KS), bf16)
    nab_h[:, :, 0:S] = (nb - 768.0).astype(bf16)[:, None, :]
    nab_h[:, :, S:] = (
        (na + 768.0).reshape(B, KS, P).transpose(0, 2, 1).astype(bf16)
    )

    # winograd x-channel weights x4096 (1/2 of G folded into dtil_1/2):
    # wtil = [g0, g0+g1+g2, g0-g1+g2, g2], layout [P, 4, KD, O]
    g = cw32[:, 0].transpose(1, 2, 0) * 4096.0          # [D, W, O]
    g0, g1, g2 = g[:, 0], g[:, 1], g[:, 2]              # [D, O]
    wt = np.stack([g0, g0 + g1 + g2, g0 - g1 + g2, g2], 0)  # [4, D, O]
    cwtw = np.ascontiguousarray(
        wt.reshape(4, KD, P, O).transpose(2, 0, 1, 3)
    ).astype(bf16).reshape(P, 4 * KD * O)
    # fused attn-channel weights Mw[w,o,t] = sum_d cw1[o,d,w] weight[t,d],
    # fp8 x32 (with attn x128 both channel groups accumulate at x4096);
    # shipped as [+mw8, -mw8] (negated copy feeds the odd-output split)
    Mw = np.einsum("odw,td->wot", cw32[:, 1], w32)
    mw8p = np.ascontiguousarray(
        (32.0 * Mw).transpose(2, 0, 1)
        .reshape(KS, P, W, O).transpose(1, 0, 2, 3)
    ).astype(f8).reshape(P, 1, KS * W * O)
    mw8 = np.concatenate([mw8p, -mw8p], axis=1).reshape(P, 2 * KS * W * O)
    cb = np.ascontiguousarray(
        np.asarray(conv_b, np.float32).reshape(MO, P).T
    )  # [P, MO]

    maps = []
    for c in range(NCORES):
        sl = slice(c * BPC, (c + 1) * BPC)
        maps.append(
            {
                "xt8a": np.ascontiguousarray(xt8a[sl]),
                "xt8b": np.ascontiguousarray(xt8b[sl]),
                "dta": np.ascontiguousarray(dta[sl]),
                "dtb": np.ascontiguousarray(dtb[sl]),
                "nab": np.ascontiguousarray(nab_h[sl]),
                "cwtw": cwtw,
                "mw8": mw8,
                "cb": cb,
            }
        )
    return maps


def _run(inputs: dict, trace: bool = False):
    nc = _build_nc()
    nc.finalize()  # Bacc.compile(): reg alloc + split multi-waits (HW max 1)
    maps = _in_maps(**inputs)
    res = run_bass_kernel_spmd(
        nc, maps, core_ids=list(range(NCORES)), trace=trace
    )
    outs = [res.results[c]["out"] for c in range(NCORES)]  # [2,BPC,O,S] bf16
    conv_a = np.concatenate(
        [np.asarray(o[0], np.float32) for o in outs], axis=0
    )
    conv_b = np.concatenate(
        [np.asarray(o[1], np.float32) for o in outs], axis=0
    )
    return (conv_a, conv_b), res


def kernel(**inputs) -> np.ndarray:
    (conv_a, conv_b), _ = _run(inputs, trace=False)
    return conv_a, conv_b
